# revision 13
# baseline (speedup 1.0000x reference)
"""DiscreteOptionActor Trainium2 kernel.

Computes, for each sample b, logits = MLP_{option[b]}(obs[b]) where each of the
16 options has its own 3-layer MLP (128 -> 256 -> 256 -> 18, ReLU).

Strategy (MoE routing):
  - Host groups samples by option (argsort) and shards options across the 8
    cores: core k handles options 2k and 2k+1. Only the selected option's trunk
    is computed per sample (16x less compute than the dense reference).
  - Per (core, option) the gathered rows are padded to PAD=4352 and stored
    transposed (feature-major [128, PAD]) so matmuls run with features on
    partitions and samples on the free (moving) dimension.
  - Device: per option, L1/L2/L3 matmuls in float32r (full-rate PE mode),
    fused bias+ReLU from PSUM on ScalarE/VectorE, output logits^T [18, PAD].
  - Host scatters results back to original row order.
"""

import numpy as np

B, OBS, OPT, H1, H2, A = 65536, 128, 16, 256, 256, 18
NCORES = 8
OPC = OPT // NCORES  # options per core = 2
PAD = 4352  # padded rows per option (multiple of 128; max count ~4216)

_CACHE = {}


def _blocks():
    out = []
    st = 0
    while st < PAD:
        nb = min(512, PAD - st)
        out.append((st, nb))
        st += nb
    return out


def _build_program():
    import concourse.bass as bass
    import concourse.bacc as bacc
    import concourse.mybir as mybir
    import concourse.tile as tile

    f32 = mybir.dt.float32
    f32r = mybir.dt.float32r
    AF = mybir.ActivationFunctionType
    ALU = mybir.AluOpType

    nc = bacc.Bacc(None, target_bir_lowering=False, debug=False)
    xt = nc.declare_dram_parameter("xt", [OPC, OBS, PAD], f32r, isOutput=False)
    w1 = nc.declare_dram_parameter("w1", [OPC, OBS, H1], f32r, isOutput=False)
    # w2/w3 pre-chunked on host to [o, p, k, n]: element [o,p,k,n] = W[o][k*128+p, n]
    w2 = nc.declare_dram_parameter("w2", [OPC, 128, 2, H2], f32r, isOutput=False)
    w3 = nc.declare_dram_parameter("w3", [OPC, 128, 2, A], f32r, isOutput=False)
    # biases host-transposed to [o, p, c]: element [o,p,c] = b[o][c*128+p]
    b1 = nc.declare_dram_parameter("b1", [OPC, 128, 2], f32, isOutput=False)
    b2 = nc.declare_dram_parameter("b2", [OPC, 128, 2], f32, isOutput=False)
    out = nc.declare_dram_parameter("out", [OPC, A, PAD], f32, isOutput=True)

    # pair-sized chunks: 4 x 1024 + 1 x 256 (PAD = 4352)
    pairs = []
    st = 0
    while st < PAD:
        nb = min(1024, PAD - st)
        pairs.append((st, nb))
        st += nb

    def halves(nb):
        out = []
        h = 0
        while h < nb:
            w = min(512, nb - h)
            out.append((h, w))
            h += w
        return out

    with tile.TileContext(nc) as tc:
        with (
            tc.tile_pool(name="wp", bufs=2) as wp,
            tc.tile_pool(name="xp", bufs=2) as xp,
            tc.tile_pool(name="hp", bufs=3) as hp,
            tc.tile_pool(name="op", bufs=3) as op,
            tc.tile_pool(name="dxp", bufs=1) as dxp,
            tc.tile_pool(name="psp", bufs=4, space=bass.MemorySpace.PSUM) as psp,
        ):
            # PE warm-up: dummy matmuls during the DMA prologue keep the
            # HAM activity monitor hot so the real stream runs at 2.4 GHz.
            dummy = dxp.tile([128, 64], f32, tag="dummy")
            nc.gpsimd.memset(dummy[:], 0.0)
            for _ in range(48):
                pw = psp.tile([128, 1024], f32, tag="ps")
                nc.tensor.matmul(
                    pw[:64, :64], dummy[:, :], dummy[:, :], start=True, stop=True
                )

            for o in range(OPC):
                # xt chunks issued first (gpsimd queue) so L1 starts early
                xtt = xp.tile([OBS, PAD], f32r, tag="xt")
                for st, nb in pairs:
                    nc.gpsimd.dma_start(xtt[:, st : st + nb], xt[o][:, st : st + nb])

                w1t = wp.tile([OBS, H1], f32r, tag="w1")
                b1t = wp.tile([128, 2], f32, tag="b1")
                w2t = wp.tile([128, 2, H2], f32r, tag="w2")
                b2t = wp.tile([128, 2], f32, tag="b2")
                w3t = wp.tile([128, 2, A], f32r, tag="w3")
                nc.sync.dma_start(w1t[:], w1[o])
                nc.sync.dma_start(b1t[:], b1[o])
                nc.sync.dma_start(b2t[:], b2[o])
                nc.sync.dma_start(w2t[:], w2[o])
                nc.sync.dma_start(w3t[:], w3[o])

                # pair-major: each 1024-col pair flows L1 -> L2 -> L3 so PE
                # can interleave later layers of pair p with DMA of pair p+1
                for st, nb in pairs:
                    h1p = hp.tile([128, 2, 1024], f32r, tag="h1")
                    h2p = hp.tile([128, 2, 1024], f32r, tag="h2")

                    # L1 (drain on ACT)
                    for c in range(2):
                        ps = psp.tile([128, 1024], f32, tag="ps")
                        for h, w in halves(nb):
                            nc.tensor.matmul(
                                ps[:, h : h + w],
                                w1t[:, c * 128 : (c + 1) * 128],
                                xtt[:, st + h : st + h + w],
                                start=True,
                                stop=True,
                            )
                        nc.scalar.activation(
                            h1p[:, c, :nb],
                            ps[:, :nb],
                            AF.Relu,
                            bias=b1t[:, c : c + 1],
                        )

                    # L2 (drain on DVE)
                    for m in range(2):
                        ps = psp.tile([128, 1024], f32, tag="ps")
                        for h, w in halves(nb):
                            for k in range(2):
                                nc.tensor.matmul(
                                    ps[:, h : h + w],
                                    w2t[:, k, m * 128 : (m + 1) * 128],
                                    h1p[:, k, h : h + w],
                                    start=(k == 0),
                                    stop=(k == 1),
                                )
                        nc.vector.tensor_scalar(
                            h2p[:, m, :nb],
                            ps[:, :nb],
                            b2t[:, m : m + 1],
                            0.0,
                            ALU.add,
                            ALU.max,
                        )

                    # L3 (drain on ACT as plain copy; b3 added on host)
                    outp = op.tile([A, 1024], f32, tag="out")
                    ps = psp.tile([A, 1024], f32, tag="ps")
                    for h, w in halves(nb):
                        for k in range(2):
                            nc.tensor.matmul(
                                ps[:, h : h + w],
                                w3t[:, k, :],
                                h2p[:, k, h : h + w],
                                start=(k == 0),
                                stop=(k == 1),
                            )
                    nc.scalar.activation(outp[:, :nb], ps[:, :nb], AF.Copy)
                    nc.sync.dma_start(out[o][:, st : st + nb], outp[:, :nb])
    nc.compile()
    return nc


def _get_program():
    if "nc" not in _CACHE:
        _CACHE["nc"] = _build_program()
    return _CACHE["nc"]


def _prep(inputs):
    obs = np.ascontiguousarray(np.asarray(inputs["obs"], dtype=np.float32))
    option = np.asarray(inputs["option"]).astype(np.int64, copy=False)
    W1 = np.asarray(inputs["W1"], dtype=np.float32)
    b1 = np.asarray(inputs["b1"], dtype=np.float32)
    W2 = np.asarray(inputs["W2"], dtype=np.float32)
    b2 = np.asarray(inputs["b2"], dtype=np.float32)
    W3 = np.asarray(inputs["W3"], dtype=np.float32)
    b3 = np.asarray(inputs["b3"], dtype=np.float32)

    order = np.argsort(option, kind="stable")
    sorted_opt = option[order]
    starts = np.searchsorted(sorted_opt, np.arange(OPT + 1))
    idx_per_opt = [order[starts[o] : starts[o + 1]] for o in range(OPT)]

    in_maps = []
    for core in range(NCORES):
        sl = slice(core * OPC, (core + 1) * OPC)
        xt = np.zeros((OPC, OBS, PAD), np.float32)
        for lo in range(OPC):
            idx = idx_per_opt[core * OPC + lo][:PAD]
            xt[lo, :, : len(idx)] = obs[idx].T
        w2c = W2[sl].reshape(OPC, 2, 128, H2).transpose(0, 2, 1, 3)
        w3c = W3[sl].reshape(OPC, 2, 128, A).transpose(0, 2, 1, 3)
        b1c = b1[sl].reshape(OPC, 2, 128).transpose(0, 2, 1)
        b2c = b2[sl].reshape(OPC, 2, 128).transpose(0, 2, 1)
        in_maps.append(
            {
                "xt": xt,
                "w1": np.ascontiguousarray(W1[sl]),
                "w2": np.ascontiguousarray(w2c),
                "w3": np.ascontiguousarray(w3c),
                "b1": np.ascontiguousarray(b1c),
                "b2": np.ascontiguousarray(b2c),
            }
        )
    host = dict(obs=obs, W1=W1, b1=b1, W2=W2, b2=b2, W3=W3, b3=b3)
    return in_maps, idx_per_opt, host


def _unshard(results, idx_per_opt, host):
    out_full = np.empty((B, 1, A), np.float32)
    for core in range(NCORES):
        res = results[core]["out"]  # [OPC, A, PAD]
        for lo in range(OPC):
            o = core * OPC + lo
            idx = idx_per_opt[o]
            n = min(len(idx), PAD)
            out_full[idx[:n], 0, :] = res[lo, :, :n].T + host["b3"][o]
            if len(idx) > n:  # overflow beyond PAD: compute on host (rare/never)
                rows = host["obs"][idx[n:]]
                h = np.maximum(rows @ host["W1"][o] + host["b1"][o], 0.0)
                h = np.maximum(h @ host["W2"][o] + host["b2"][o], 0.0)
                out_full[idx[n:], 0, :] = h @ host["W3"][o] + host["b3"][o]
    return out_full


def run(inputs, trace=False, **spmd_kwargs):
    """Run the kernel; returns (output, BassKernelResults)."""
    from concourse.bass_utils import run_bass_kernel_spmd

    in_maps, idx_per_opt, host = _prep(inputs)
    nc = _get_program()
    br = run_bass_kernel_spmd(
        nc, in_maps, list(range(NCORES)), trace=trace, **spmd_kwargs
    )
    return _unshard(br.results, idx_per_opt, host), br


def kernel(**inputs):
    out, _ = run(inputs)
    return out


# revision 15
# speedup vs baseline: 1.0993x; 1.0993x over previous
"""DiscreteOptionActor Trainium2 kernel.

Computes, for each sample b, logits = MLP_{option[b]}(obs[b]) where each of the
16 options has its own 3-layer MLP (128 -> 256 -> 256 -> 18, ReLU).

Strategy (MoE routing):
  - Host groups samples by option (argsort) and shards options across the 8
    cores: core k handles options 2k and 2k+1. Only the selected option's trunk
    is computed per sample (16x less compute than the dense reference).
  - Per (core, option) the gathered rows are padded to PAD=4352 and stored
    transposed (feature-major [128, PAD]) so matmuls run with features on
    partitions and samples on the free (moving) dimension.
  - Device: per option, L1/L2/L3 matmuls in float32r (full-rate PE mode),
    fused bias+ReLU from PSUM on ScalarE/VectorE, output logits^T [18, PAD].
  - Host scatters results back to original row order.
"""

import numpy as np

B, OBS, OPT, H1, H2, A = 65536, 128, 16, 256, 256, 18
NCORES = 8
OPC = OPT // NCORES  # options per core = 2
PAD = 4352  # padded rows per option (multiple of 128; max count ~4216)

_CACHE = {}


def _blocks():
    out = []
    st = 0
    while st < PAD:
        nb = min(512, PAD - st)
        out.append((st, nb))
        st += nb
    return out


def _build_program():
    import concourse.bass as bass
    import concourse.bacc as bacc
    import concourse.mybir as mybir
    import concourse.tile as tile

    f32 = mybir.dt.float32
    f32r = mybir.dt.float32r
    AF = mybir.ActivationFunctionType
    ALU = mybir.AluOpType

    nc = bacc.Bacc(None, target_bir_lowering=False, debug=False)
    xt = nc.declare_dram_parameter("xt", [OPC, OBS, PAD], f32r, isOutput=False)
    w1 = nc.declare_dram_parameter("w1", [OPC, OBS, H1], f32r, isOutput=False)
    # w2/w3 pre-chunked on host to [o, p, k, n]: element [o,p,k,n] = W[o][k*128+p, n]
    w2 = nc.declare_dram_parameter("w2", [OPC, 128, 2, H2], f32r, isOutput=False)
    w3 = nc.declare_dram_parameter("w3", [OPC, 128, 2, A], f32r, isOutput=False)
    # biases host-transposed to [o, p, c]: element [o,p,c] = b[o][c*128+p]
    b1 = nc.declare_dram_parameter("b1", [OPC, 128, 2], f32, isOutput=False)
    b2 = nc.declare_dram_parameter("b2", [OPC, 128, 2], f32, isOutput=False)
    out = nc.declare_dram_parameter("out", [OPC, A, PAD], f32, isOutput=True)

    # pair-sized chunks: 4 x 1024 + 1 x 256 (PAD = 4352)
    pairs = []
    st = 0
    while st < PAD:
        nb = min(1024, PAD - st)
        pairs.append((st, nb))
        st += nb

    def halves(nb):
        out = []
        h = 0
        while h < nb:
            w = min(512, nb - h)
            out.append((h, w))
            h += w
        return out

    with tile.TileContext(nc) as tc:
        with (
            tc.tile_pool(name="wp", bufs=2) as wp,
            tc.tile_pool(name="xp", bufs=2) as xp,
            tc.tile_pool(name="hp1", bufs=3) as hp1,
            tc.tile_pool(name="hp2", bufs=3) as hp2,
            tc.tile_pool(name="op", bufs=3) as op,
            tc.tile_pool(name="dxp", bufs=1) as dxp,
            tc.tile_pool(name="psp", bufs=4, space=bass.MemorySpace.PSUM) as psp,
        ):
            # PE warm-up: dummy matmuls during the DMA prologue keep the
            # HAM activity monitor hot so the real stream runs at 2.4 GHz.
            dummy = dxp.tile([128, 64], f32, tag="dummy")
            nc.gpsimd.memset(dummy[:], 0.0)
            for _ in range(36):
                pw = psp.tile([128, 1024], f32, tag="ps")
                nc.tensor.matmul(
                    pw[:64, :64], dummy[:, :], dummy[:, :], start=True, stop=True
                )

            for o in range(OPC):
                # xt chunks issued first (gpsimd queue) so L1 starts early
                xtt = xp.tile([OBS, PAD], f32r, tag="xt")
                for st, nb in pairs:
                    nc.gpsimd.dma_start(xtt[:, st : st + nb], xt[o][:, st : st + nb])

                w1t = wp.tile([OBS, H1], f32r, tag="w1")
                b1t = wp.tile([128, 2], f32, tag="b1")
                w2t = wp.tile([128, 2, H2], f32r, tag="w2")
                b2t = wp.tile([128, 2], f32, tag="b2")
                w3t = wp.tile([128, 2, A], f32r, tag="w3")
                nc.sync.dma_start(w1t[:], w1[o])
                nc.sync.dma_start(b1t[:], b1[o])
                nc.sync.dma_start(b2t[:], b2[o])
                nc.sync.dma_start(w2t[:], w2[o])
                nc.sync.dma_start(w3t[:], w3[o])

                h1c = [hp1.tile([128, PAD], f32r, tag="h1", name=f"h1c{_c}") for _c in range(2)]
                h2c = [hp2.tile([128, PAD], f32r, tag="h2", name=f"h2c{_c}") for _c in range(2)]

                # L1: h1^T = relu(W1^T x + b1) -> ACT
                for c in range(2):
                    for st, nb in pairs:
                        ps = psp.tile([128, 1024], f32, tag="ps")
                        for h, w in halves(nb):
                            nc.tensor.matmul(
                                ps[:, h : h + w],
                                w1t[:, c * 128 : (c + 1) * 128],
                                xtt[:, st + h : st + h + w],
                                start=True,
                                stop=True,
                            )
                        nc.scalar.activation(
                            h1c[c][:, st : st + nb],
                            ps[:, :nb],
                            AF.Relu,
                            bias=b1t[:, c : c + 1],
                        )

                # L2: drain on DVE
                for m in range(2):
                    for st, nb in pairs:
                        ps = psp.tile([128, 1024], f32, tag="ps")
                        for h, w in halves(nb):
                            for k in range(2):
                                nc.tensor.matmul(
                                    ps[:, h : h + w],
                                    w2t[:, k, m * 128 : (m + 1) * 128],
                                    h1c[k][:, st + h : st + h + w],
                                    start=(k == 0),
                                    stop=(k == 1),
                                )
                        nc.vector.tensor_scalar(
                            h2c[m][:, st : st + nb],
                            ps[:, :nb],
                            b2t[:, m : m + 1],
                            0.0,
                            ALU.add,
                            ALU.max,
                        )

                # L3: drain on ACT as plain copy (b3 added on host), chunked out
                for st, nb in pairs:
                    outp = op.tile([A, 1024], f32, tag="out")
                    ps = psp.tile([A, 1024], f32, tag="ps")
                    for h, w in halves(nb):
                        for k in range(2):
                            nc.tensor.matmul(
                                ps[:, h : h + w],
                                w3t[:, k, :],
                                h2c[k][:, st + h : st + h + w],
                                start=(k == 0),
                                stop=(k == 1),
                            )
                    nc.scalar.activation(outp[:, :nb], ps[:, :nb], AF.Copy)
                    nc.sync.dma_start(out[o][:, st : st + nb], outp[:, :nb])
    nc.compile()
    return nc


def _get_program():
    if "nc" not in _CACHE:
        _CACHE["nc"] = _build_program()
    return _CACHE["nc"]


def _prep(inputs):
    obs = np.ascontiguousarray(np.asarray(inputs["obs"], dtype=np.float32))
    option = np.asarray(inputs["option"]).astype(np.int64, copy=False)
    W1 = np.asarray(inputs["W1"], dtype=np.float32)
    b1 = np.asarray(inputs["b1"], dtype=np.float32)
    W2 = np.asarray(inputs["W2"], dtype=np.float32)
    b2 = np.asarray(inputs["b2"], dtype=np.float32)
    W3 = np.asarray(inputs["W3"], dtype=np.float32)
    b3 = np.asarray(inputs["b3"], dtype=np.float32)

    order = np.argsort(option, kind="stable")
    sorted_opt = option[order]
    starts = np.searchsorted(sorted_opt, np.arange(OPT + 1))
    idx_per_opt = [order[starts[o] : starts[o + 1]] for o in range(OPT)]

    in_maps = []
    for core in range(NCORES):
        sl = slice(core * OPC, (core + 1) * OPC)
        xt = np.zeros((OPC, OBS, PAD), np.float32)
        for lo in range(OPC):
            idx = idx_per_opt[core * OPC + lo][:PAD]
            xt[lo, :, : len(idx)] = obs[idx].T
        w2c = W2[sl].reshape(OPC, 2, 128, H2).transpose(0, 2, 1, 3)
        w3c = W3[sl].reshape(OPC, 2, 128, A).transpose(0, 2, 1, 3)
        b1c = b1[sl].reshape(OPC, 2, 128).transpose(0, 2, 1)
        b2c = b2[sl].reshape(OPC, 2, 128).transpose(0, 2, 1)
        in_maps.append(
            {
                "xt": xt,
                "w1": np.ascontiguousarray(W1[sl]),
                "w2": np.ascontiguousarray(w2c),
                "w3": np.ascontiguousarray(w3c),
                "b1": np.ascontiguousarray(b1c),
                "b2": np.ascontiguousarray(b2c),
            }
        )
    host = dict(obs=obs, W1=W1, b1=b1, W2=W2, b2=b2, W3=W3, b3=b3)
    return in_maps, idx_per_opt, host


def _unshard(results, idx_per_opt, host):
    out_full = np.empty((B, 1, A), np.float32)
    for core in range(NCORES):
        res = results[core]["out"]  # [OPC, A, PAD]
        for lo in range(OPC):
            o = core * OPC + lo
            idx = idx_per_opt[o]
            n = min(len(idx), PAD)
            out_full[idx[:n], 0, :] = res[lo, :, :n].T + host["b3"][o]
            if len(idx) > n:  # overflow beyond PAD: compute on host (rare/never)
                rows = host["obs"][idx[n:]]
                h = np.maximum(rows @ host["W1"][o] + host["b1"][o], 0.0)
                h = np.maximum(h @ host["W2"][o] + host["b2"][o], 0.0)
                out_full[idx[n:], 0, :] = h @ host["W3"][o] + host["b3"][o]
    return out_full


def run(inputs, trace=False, **spmd_kwargs):
    """Run the kernel; returns (output, BassKernelResults)."""
    from concourse.bass_utils import run_bass_kernel_spmd

    in_maps, idx_per_opt, host = _prep(inputs)
    nc = _get_program()
    br = run_bass_kernel_spmd(
        nc, in_maps, list(range(NCORES)), trace=trace, **spmd_kwargs
    )
    return _unshard(br.results, idx_per_opt, host), br


def kernel(**inputs):
    out, _ = run(inputs)
    return out


# revision 17
# speedup vs baseline: 1.1590x; 1.0543x over previous
"""DiscreteOptionActor Trainium2 kernel.

Computes, for each sample b, logits = MLP_{option[b]}(obs[b]) where each of the
16 options has its own 3-layer MLP (128 -> 256 -> 256 -> 18, ReLU).

Strategy (MoE routing):
  - Host groups samples by option (argsort) and shards options across the 8
    cores: core k handles options 2k and 2k+1. Only the selected option's trunk
    is computed per sample (16x less compute than the dense reference).
  - Per (core, option) the gathered rows are padded to PAD=4352 and stored
    transposed (feature-major [128, PAD]) so matmuls run with features on
    partitions and samples on the free (moving) dimension.
  - Device: per option, L1/L2/L3 matmuls in float32r (full-rate PE mode),
    fused bias+ReLU from PSUM on ScalarE/VectorE, output logits^T [18, PAD].
  - Host scatters results back to original row order.
"""

import numpy as np

B, OBS, OPT, H1, H2, A = 65536, 128, 16, 256, 256, 18
NCORES = 8
OPC = OPT // NCORES  # options per core = 2
PAD = 4352  # padded rows per option (multiple of 128; max count ~4216)

_CACHE = {}


def _blocks():
    out = []
    st = 0
    while st < PAD:
        nb = min(512, PAD - st)
        out.append((st, nb))
        st += nb
    return out


def _build_program():
    import concourse.bass as bass
    import concourse.bacc as bacc
    import concourse.mybir as mybir
    import concourse.tile as tile

    f32 = mybir.dt.float32
    f32r = mybir.dt.float32r
    AF = mybir.ActivationFunctionType
    ALU = mybir.AluOpType

    nc = bacc.Bacc(None, target_bir_lowering=False, debug=False)
    xt = nc.declare_dram_parameter("xt", [OPC, OBS, PAD], f32r, isOutput=False)
    w1 = nc.declare_dram_parameter("w1", [OPC, OBS, H1], f32r, isOutput=False)
    # w2/w3 pre-chunked on host to [o, p, k, n]: element [o,p,k,n] = W[o][k*128+p, n]
    w2 = nc.declare_dram_parameter("w2", [OPC, 128, 2, H2], f32r, isOutput=False)
    w3 = nc.declare_dram_parameter("w3", [OPC, 128, 2, A], f32r, isOutput=False)
    # biases host-transposed to [o, p, c]: element [o,p,c] = b[o][c*128+p]
    b1 = nc.declare_dram_parameter("b1", [OPC, 128, 2], f32, isOutput=False)
    b2 = nc.declare_dram_parameter("b2", [OPC, 128, 2], f32, isOutput=False)
    out = nc.declare_dram_parameter("out", [OPC, A, PAD], f32, isOutput=True)

    # pair-sized chunks: 4 x 1024 + 1 x 256 (PAD = 4352)
    pairs = []
    st = 0
    while st < PAD:
        nb = min(1024, PAD - st)
        pairs.append((st, nb))
        st += nb

    def halves(nb):
        out = []
        h = 0
        while h < nb:
            w = min(512, nb - h)
            out.append((h, w))
            h += w
        return out

    with tile.TileContext(nc) as tc:
        with (
            tc.tile_pool(name="wp", bufs=2) as wp,
            tc.tile_pool(name="xp", bufs=2) as xp,
            tc.tile_pool(name="hp1", bufs=3) as hp1,
            tc.tile_pool(name="hp2", bufs=3) as hp2,
            tc.tile_pool(name="op", bufs=3) as op,
            tc.tile_pool(name="dxp", bufs=1) as dxp,
            tc.tile_pool(name="psp", bufs=4, space=bass.MemorySpace.PSUM) as psp,
        ):
            # smaller leading chunks so L1 can start as early as possible
            xchunks = [(0, 512), (512, 512), (1024, 1024), (2048, 1024), (3072, 1024), (4096, 256)]
            assert sum(nb for _, nb in xchunks) == PAD

            xtts = []
            for o in range(OPC):
                xtt = xp.tile([OBS, PAD], f32r, tag="xt", name=f"xtt{o}")
                xtts.append(xtt)
            # first data chunk issued before anything else on gpsimd
            nc.gpsimd.dma_start(
                xtts[0][:, 0:512], xt[0][:, 0:512]
            )

            # PE warm-up: dummy matmuls during the DMA prologue keep the
            # HAM activity monitor hot so the real stream runs at 2.4 GHz.
            dummy = dxp.tile([128, 64], f32, tag="dummy")
            nc.gpsimd.memset(dummy[:], 0.0)
            for _ in range(36):
                pw = psp.tile([128, 1024], f32, tag="ps")
                nc.tensor.matmul(
                    pw[:64, :64], dummy[:, :], dummy[:, :], start=True, stop=True
                )

            for o in range(OPC):
                # xt chunks issued first (gpsimd queue) so L1 starts early
                xtt = xtts[o]
                for st, nb in xchunks:
                    if o == 0 and st == 0:
                        continue  # already issued above
                    nc.gpsimd.dma_start(xtt[:, st : st + nb], xt[o][:, st : st + nb])

                w1t = wp.tile([OBS, H1], f32r, tag="w1")
                b1t = wp.tile([128, 2], f32, tag="b1")
                w2t = wp.tile([128, 2, H2], f32r, tag="w2")
                b2t = wp.tile([128, 2], f32, tag="b2")
                w3t = wp.tile([128, 2, A], f32r, tag="w3")
                nc.sync.dma_start(w1t[:], w1[o])
                nc.sync.dma_start(b1t[:], b1[o])
                nc.sync.dma_start(b2t[:], b2[o])
                nc.sync.dma_start(w2t[:], w2[o])
                nc.sync.dma_start(w3t[:], w3[o])

                h1c = [hp1.tile([128, PAD], f32r, tag="h1", name=f"h1c{_c}") for _c in range(2)]
                h2c = [hp2.tile([128, PAD], f32r, tag="h2", name=f"h2c{_c}") for _c in range(2)]

                # Drains alternate ACT/DVE so the PSUM rotation is never
                # limited by a single drain engine in any phase.
                def relu_drain(dst, ps_ap, bias_ap, use_act):
                    if use_act:
                        nc.scalar.activation(dst, ps_ap, AF.Relu, bias=bias_ap)
                    else:
                        nc.vector.tensor_scalar(
                            dst, ps_ap, bias_ap, 0.0, ALU.add, ALU.max
                        )

                # L1: h1^T = relu(W1^T x + b1)
                di = 0
                for c in range(2):
                    for st, nb in pairs:
                        ps = psp.tile([128, 1024], f32, tag="ps")
                        for h, w in halves(nb):
                            nc.tensor.matmul(
                                ps[:, h : h + w],
                                w1t[:, c * 128 : (c + 1) * 128],
                                xtt[:, st + h : st + h + w],
                                start=True,
                                stop=True,
                            )
                        relu_drain(
                            h1c[c][:, st : st + nb],
                            ps[:, :nb],
                            b1t[:, c : c + 1],
                            di % 2 == 0,
                        )
                        di += 1

                # L2
                for m in range(2):
                    for st, nb in pairs:
                        ps = psp.tile([128, 1024], f32, tag="ps")
                        for h, w in halves(nb):
                            for k in range(2):
                                nc.tensor.matmul(
                                    ps[:, h : h + w],
                                    w2t[:, k, m * 128 : (m + 1) * 128],
                                    h1c[k][:, st + h : st + h + w],
                                    start=(k == 0),
                                    stop=(k == 1),
                                )
                        relu_drain(
                            h2c[m][:, st : st + nb],
                            ps[:, :nb],
                            b2t[:, m : m + 1],
                            di % 2 == 0,
                        )
                        di += 1

                # L3: plain copy drain (b3 added on host), chunked out DMA
                for st, nb in pairs:
                    outp = op.tile([A, 1024], f32, tag="out")
                    ps = psp.tile([A, 1024], f32, tag="ps")
                    for h, w in halves(nb):
                        for k in range(2):
                            nc.tensor.matmul(
                                ps[:, h : h + w],
                                w3t[:, k, :],
                                h2c[k][:, st + h : st + h + w],
                                start=(k == 0),
                                stop=(k == 1),
                            )
                    if di % 2 == 0:
                        nc.scalar.activation(outp[:, :nb], ps[:, :nb], AF.Copy)
                    else:
                        nc.vector.tensor_copy(outp[:, :nb], ps[:, :nb])
                    di += 1
                    nc.sync.dma_start(out[o][:, st : st + nb], outp[:, :nb])
    nc.compile()
    return nc


def _get_program():
    if "nc" not in _CACHE:
        _CACHE["nc"] = _build_program()
    return _CACHE["nc"]


def _prep(inputs):
    obs = np.ascontiguousarray(np.asarray(inputs["obs"], dtype=np.float32))
    option = np.asarray(inputs["option"]).astype(np.int64, copy=False)
    W1 = np.asarray(inputs["W1"], dtype=np.float32)
    b1 = np.asarray(inputs["b1"], dtype=np.float32)
    W2 = np.asarray(inputs["W2"], dtype=np.float32)
    b2 = np.asarray(inputs["b2"], dtype=np.float32)
    W3 = np.asarray(inputs["W3"], dtype=np.float32)
    b3 = np.asarray(inputs["b3"], dtype=np.float32)

    order = np.argsort(option, kind="stable")
    sorted_opt = option[order]
    starts = np.searchsorted(sorted_opt, np.arange(OPT + 1))
    idx_per_opt = [order[starts[o] : starts[o + 1]] for o in range(OPT)]

    in_maps = []
    for core in range(NCORES):
        sl = slice(core * OPC, (core + 1) * OPC)
        xt = np.zeros((OPC, OBS, PAD), np.float32)
        for lo in range(OPC):
            idx = idx_per_opt[core * OPC + lo][:PAD]
            xt[lo, :, : len(idx)] = obs[idx].T
        w2c = W2[sl].reshape(OPC, 2, 128, H2).transpose(0, 2, 1, 3)
        w3c = W3[sl].reshape(OPC, 2, 128, A).transpose(0, 2, 1, 3)
        b1c = b1[sl].reshape(OPC, 2, 128).transpose(0, 2, 1)
        b2c = b2[sl].reshape(OPC, 2, 128).transpose(0, 2, 1)
        in_maps.append(
            {
                "xt": xt,
                "w1": np.ascontiguousarray(W1[sl]),
                "w2": np.ascontiguousarray(w2c),
                "w3": np.ascontiguousarray(w3c),
                "b1": np.ascontiguousarray(b1c),
                "b2": np.ascontiguousarray(b2c),
            }
        )
    host = dict(obs=obs, W1=W1, b1=b1, W2=W2, b2=b2, W3=W3, b3=b3)
    return in_maps, idx_per_opt, host


def _unshard(results, idx_per_opt, host):
    out_full = np.empty((B, 1, A), np.float32)
    for core in range(NCORES):
        res = results[core]["out"]  # [OPC, A, PAD]
        for lo in range(OPC):
            o = core * OPC + lo
            idx = idx_per_opt[o]
            n = min(len(idx), PAD)
            out_full[idx[:n], 0, :] = res[lo, :, :n].T + host["b3"][o]
            if len(idx) > n:  # overflow beyond PAD: compute on host (rare/never)
                rows = host["obs"][idx[n:]]
                h = np.maximum(rows @ host["W1"][o] + host["b1"][o], 0.0)
                h = np.maximum(h @ host["W2"][o] + host["b2"][o], 0.0)
                out_full[idx[n:], 0, :] = h @ host["W3"][o] + host["b3"][o]
    return out_full


def run(inputs, trace=False, **spmd_kwargs):
    """Run the kernel; returns (output, BassKernelResults)."""
    from concourse.bass_utils import run_bass_kernel_spmd

    in_maps, idx_per_opt, host = _prep(inputs)
    nc = _get_program()
    br = run_bass_kernel_spmd(
        nc, in_maps, list(range(NCORES)), trace=trace, **spmd_kwargs
    )
    return _unshard(br.results, idx_per_opt, host), br


def kernel(**inputs):
    out, _ = run(inputs)
    return out
